# revision 1
# baseline (speedup 1.0000x reference)
"""DGCNN generator kernel for Trainium2 (Bass/Tile), data-parallel over 8 NeuronCores.

Each core processes one point cloud (B=8). Per conv layer:
  - pairwise-distance surrogate S = X^T X - sq_j/2 via PE matmuls (K=65 augmented)
  - exact top-10 neighbor indices per point via DVE max8/max_index
  - neighbor feature gather via GPSIMD ap_gather (channel-on-partition layout)
  - edge MLP relu(A_i + B_j) @ w2, max over the 10 neighbors
Point order is re-permuted every conv (sigma(c) = 16*(c%128) + c//128) to make the
index-relayout DMAs affine; all downstream stages are permutation-invariant.
"""
import sys
sys.path.insert(0, "/opt/trn_rl_repo")

import numpy as np
from contextlib import ExitStack

import concourse.bass as bass
import concourse.bacc as bacc
import concourse.tile as tile
from concourse.tile import add_dep_helper
from concourse import mybir
from concourse.bass_utils import run_bass_kernel_spmd

FP = mybir.dt.float32
U16 = mybir.dt.uint16
I16 = mybir.dt.int16

NPTS = 2048
NCORES = 8
KNN = 10
NT = 16            # 128-row tiles per conv
DOUT = [64, 64, 64, 128]
NEG = -1e30

# matmul dtype for the big matmuls (switched to float32r if numerics allow)
MM_DT = mybir.dt.float32


def _mm(nc, out, lhsT, rhs, **kw):
    if MM_DT == mybir.dt.float32:
        return nc.tensor.matmul(out, lhsT, rhs, **kw)
    return nc.tensor.matmul(out, lhsT.bitcast(MM_DT), rhs.bitcast(MM_DT), **kw)


def build_program():
    nc = bacc.Bacc("TRN2", target_bir_lowering=False, debug=False,
                   enable_asserts=False, num_devices=NCORES)

    # ---- per-core inputs
    posT = nc.dram_tensor("posT", [64, NPTS], FP, kind="ExternalInput").ap()
    ident = nc.dram_tensor("ident", [128, 128], FP, kind="ExternalInput").ap()
    emb = nc.dram_tensor("emb", [64, 1], FP, kind="ExternalInput").ap()
    # ---- shared weights
    w1d, w1b, b1, w2, b2 = [], [], [], [], []
    for l in range(4):
        d = DOUT[l]
        w1d.append(nc.dram_tensor(f"w1d{l}", [64, d], FP, kind="ExternalInput").ap())
        w1b.append(nc.dram_tensor(f"w1b{l}", [64, d], FP, kind="ExternalInput").ap())
        b1.append(nc.dram_tensor(f"b1{l}", [d, 1], FP, kind="ExternalInput").ap())
        w2.append(nc.dram_tensor(f"w2{l}", [d, d], FP, kind="ExternalInput").ap())
        b2.append(nc.dram_tensor(f"b2{l}", [d, 1], FP, kind="ExternalInput").ap())
    encw = nc.dram_tensor("encw", [384, 512], FP, kind="ExternalInput").ap()
    encb = nc.dram_tensor("encb", [1, 512], FP, kind="ExternalInput").ap()
    decw1 = nc.dram_tensor("decw1", [512, 1024], FP, kind="ExternalInput").ap()
    decb1 = nc.dram_tensor("decb1", [1, 1024], FP, kind="ExternalInput").ap()
    decw2 = nc.dram_tensor("decw2", [1024, 3072], FP, kind="ExternalInput").ap()
    decb2 = nc.dram_tensor("decb2", [1, 3072], FP, kind="ExternalInput").ap()
    out = nc.dram_tensor("out", [1024, 3], FP, kind="ExternalOutput").ap()

    io = dict(posT=posT, emb=emb, encw=encw, encb=encb, decw1=decw1,
              decb1=decb1, decw2=decw2, decb2=decb2, out=out, ident=ident)
    for l in range(4):
        io[f"w1d{l}"], io[f"w1b{l}"], io[f"b1{l}"] = w1d[l], w1b[l], b1[l]
        io[f"w2{l}"], io[f"b2{l}"] = w2[l], b2[l]
    with tile.TileContext(nc) as tc, ExitStack() as ctx:
        _body(ctx, tc, io)
    nc.compile()
    return nc


def _body(ctx, tc, io):
    nc = tc.nc
    posT, emb, out = io["posT"], io["emb"], io["out"]
    ident_d = io["ident"]
    encw, encb = io["encw"], io["encb"]
    decw1, decb1, decw2, decb2 = io["decw1"], io["decb1"], io["decw2"], io["decb2"]
    w1d = [io[f"w1d{l}"] for l in range(4)]
    w1b = [io[f"w1b{l}"] for l in range(4)]
    b1 = [io[f"b1{l}"] for l in range(4)]
    w2 = [io[f"w2{l}"] for l in range(4)]
    b2 = [io[f"b2{l}"] for l in range(4)]

    persist = ctx.enter_context(tc.tile_pool(name="persist", bufs=1))

    # ping-pong feature tensors: rows 0:64 features, row 64 aug
    xa = [persist.tile([65, NPTS], FP, tag=f"xa{i}", name=f"xa{i}") for i in range(2)]
    xb = [persist.tile([65, NPTS], FP, tag=f"xb{i}", name=f"xb{i}") for i in range(2)]
    for i in range(2):
        nc.vector.memset(xa[i][64:65, :], 1.0)   # ones row (never overwritten)
    xa_writer = [None, None]
    xa_writer[0] = nc.sync.dma_start(xa[0][0:64, :], posT[:])
    nc.sync.dma_start(xb[0][0:64, :], posT[:])

    ones64 = persist.tile([64, 1], FP)
    nc.vector.memset(ones64[:], 1.0)
    ident = persist.tile([128, 128], FP)
    nc.sync.dma_start(ident[:], ident_d[:])

    gtiles = []   # per-conv global-max [d, 1]
    x4_final = None

    for l in range(4):
        d = DOUT[l]
        cur_a, cur_b = xa[l % 2], xb[l % 2]
        with ExitStack() as cctx:
            wpool = cctx.enter_context(tc.tile_pool(name=f"w{l}", bufs=1))
            # load weights for this conv
            w1d_sb = wpool.tile([64, d], FP)
            nc.sync.dma_start(w1d_sb[:], w1d[l][:])
            w1b_sb = wpool.tile([64, d], FP)
            nc.sync.dma_start(w1b_sb[:], w1b[l][:])
            b1_sb = wpool.tile([d, 1], FP)
            nc.sync.dma_start(b1_sb[:], b1[l][:])
            w2_sb = wpool.tile([d, d], FP)
            nc.sync.dma_start(w2_sb[:], w2[l][:])
            b2_sb = wpool.tile([d, 1], FP)
            nc.sync.dma_start(b2_sb[:], b2[l][:])

            # ---- -sq/2 into cur_b row 64
            with tc.tile_pool(name=f"sq{l}", bufs=2, space="PSUM") as ps_sq, \
                 tc.tile_pool(name=f"sqs{l}", bufs=2) as sq_sb:
                x2 = sq_sb.tile([64, NPTS], FP, tag="x2")
                nc.vector.scalar_tensor_tensor(
                    out=x2[:], in0=cur_a[0:64, :], scalar=-0.5, in1=cur_a[0:64, :],
                    op0=mybir.AluOpType.mult, op1=mybir.AluOpType.mult)
                for j in range(4):
                    sq_ps = ps_sq.tile([1, 512], FP, tag="sq")
                    _mm(nc, sq_ps[:], ones64[:], x2[:, 512 * j: 512 * (j + 1)],
                        start=True, stop=True)
                    nc.scalar.copy(cur_b[64:65, 512 * j: 512 * (j + 1)], sq_ps[:])

            # ---- phase B: distances + topk per 128-row tile
            iia = persist.tile([128, 128], U16, name=f"iia{l}")
            iib = persist.tile([128, 128], U16, name=f"iib{l}")
            nc.vector.memset(iia[:], 0)
            nc.vector.memset(iib[:], 0)
            with tc.tile_pool(name=f"s{l}", bufs=2, space="PSUM") as ps_s, \
                 tc.tile_pool(name=f"c{l}", bufs=3) as cpool:
                for m in range(NT):
                    s_ps = ps_s.tile([128, NPTS], FP, tag="s")
                    for j in range(4):
                        _mm(nc, s_ps[:, 512 * j: 512 * (j + 1)],
                            cur_a[:, 128 * m: 128 * (m + 1)],
                            cur_b[:, 512 * j: 512 * (j + 1)], start=True, stop=True)
                    c = cpool.tile([128, 64], FP, tag="c")
                    for ch in range(8):
                        nc.vector.max(c[:, 8 * ch: 8 * ch + 8],
                                      s_ps[:, 256 * ch: 256 * (ch + 1)])
                    v1 = cpool.tile([128, 8], FP, tag="v1")
                    nc.vector.max(v1[:], c[:])
                    c2 = cpool.tile([128, 64], FP, tag="c2")
                    nc.vector.match_replace(c2[:], v1[:], c[:], NEG)
                    v2 = cpool.tile([128, 8], FP, tag="v2")
                    nc.vector.max(v2[:], c2[:])
                    ii = iia if m < 8 else iib
                    t = m % 8
                    nc.vector.max_index(ii[:, 16 * t: 16 * t + 8], v1[:], s_ps[:])
                    nc.vector.max_index(ii[:, 16 * t + 8: 16 * t + 16], v2[:], s_ps[:])

            # ---- transpose + relayout into IDXW
            idxw = persist.tile([128, 1280], U16, name=f"idxw{l}")
            # s-major fp32 reorder -> PE transpose -> u16; contiguous relays
            Ts = [persist.tile([128, 128], U16, tag=f"T{a}", name=f"T{l}{a}") for a in range(2)]
            with tc.tile_pool(name=f"tp{l}", bufs=2, space="PSUM") as ps_t, \
                 tc.tile_pool(name=f"tf{l}", bufs=2) as tfp:
                for a, ii in enumerate([iia, iib]):
                    iif = tfp.tile([128, 128], FP, tag="iif")
                    nc.vector.tensor_copy(
                        iif[:].rearrange("p (s t) -> p t s", s=16, t=8),
                        ii[:].rearrange("p (t s) -> p t s", t=8, s=16))
                    tp = ps_t.tile([128, 128], FP, tag="tp")
                    nc.tensor.transpose(tp[:], iif[:], ident[:])
                    nc.vector.tensor_copy(Ts[a][:], tp[:])
            isem = nc.alloc_semaphore(f"idxsem{l}")
            with tc.tile_critical():
                for a in range(2):
                    for k in range(KNN):
                        nc.sync.dma_start(
                            idxw[8 * a: 8 * a + 8, 128 * k: 128 * (k + 1)],
                            Ts[a][8 * k: 8 * k + 8, :]).then_inc(isem, 16)
                for rc in range(1, 8):
                    nc.sync.dma_start(idxw[16 * rc: 16 * rc + 16, :],
                                      idxw[0:16, :])._wait_ge(isem, 320).then_inc(isem, 16)
                nc.sync.nop()._wait_ge(isem, 432)

            # ---- phase C: A (sigma-permuted), Bt, gather
            bt = wpool.tile([d, NPTS], FP, name=f"bt{l}")
            a_sb = wpool.tile([d, NPTS], FP, name=f"a{l}")
            with tc.tile_pool(name=f"ab{l}", bufs=2, space="PSUM") as ps_ab:
                for j in range(4):
                    b_ps = ps_ab.tile([d, 512], FP, tag="ab")
                    _mm(nc, b_ps[:], w1b_sb[:],
                        cur_a[0:64, 512 * j: 512 * (j + 1)], start=True, stop=True)
                    nc.scalar.copy(bt[:, 512 * j: 512 * (j + 1)], b_ps[:])
                for j in range(4):
                    a_ps = ps_ab.tile([d, 512], FP, tag="ab")
                    rhs = cur_a[0:64, :].rearrange(
                        "c (q p) -> c p q", q=16, p=128)[:, 32 * j: 32 * (j + 1), :]
                    amm = _mm(nc, a_ps[:], w1d_sb[:], rhs, start=True, stop=True)
                    if xa_writer[l % 2] is not None and amm is not None:
                        add_dep_helper(amm.ins, xa_writer[l % 2].ins,
                                       reason="perm-rhs read after xa write")
                    nc.vector.tensor_scalar_add(
                        a_sb[:, 512 * j: 512 * (j + 1)], a_ps[:], b1_sb[:])

            bg = wpool.tile([d, KNN * NPTS], FP, name=f"bg{l}")
            nc.gpsimd.ap_gather(bg[:], bt[:], idxw[0:d, :].bitcast(I16),
                                channels=d, num_elems=NPTS, d=1,
                                num_idxs=KNN * NPTS)

            # ---- phase D/E: edge MLP + max over k
            zmax = wpool.tile([d, NPTS], FP, name=f"zmax{l}")
            with tc.tile_pool(name=f"z{l}", bufs=2, space="PSUM") as ps_z, \
                 tc.tile_pool(name=f"h{l}", bufs=2) as hpool:
                for k in range(KNN):
                    h1 = hpool.tile([d, NPTS], FP, tag="h1")
                    nc.vector.tensor_tensor(
                        h1[:], a_sb[:], bg[:, NPTS * k: NPTS * (k + 1)],
                        mybir.AluOpType.add)
                    h1r = hpool.tile([d, NPTS], FP, tag="h1r")
                    nc.scalar.activation(h1r[:], h1[:],
                                         mybir.ActivationFunctionType.Relu)
                    for jh in range(2):
                        z_ps = ps_z.tile([d, 1024], FP, tag="z")
                        for jj in range(2):
                            _mm(nc, z_ps[:, 512 * jj: 512 * (jj + 1)], w2_sb[:],
                                h1r[:, 1024 * jh + 512 * jj: 1024 * jh + 512 * (jj + 1)],
                                start=True, stop=True)
                        if k == 0:
                            nc.scalar.copy(zmax[:, 1024 * jh: 1024 * (jh + 1)], z_ps[:])
                        else:
                            nc.vector.tensor_tensor(
                                zmax[:, 1024 * jh: 1024 * (jh + 1)],
                                zmax[:, 1024 * jh: 1024 * (jh + 1)], z_ps[:],
                                mybir.AluOpType.max)

            # ---- x_next = zmax + b2 ; g_l
            if l < 3:
                nxt_a, nxt_b = xa[(l + 1) % 2], xb[(l + 1) % 2]
                xa_writer[(l + 1) % 2] = nc.vector.tensor_scalar_add(
                    nxt_a[0:64, :], zmax[:], b2_sb[:])
                nc.scalar.copy(nxt_b[0:64, :], nxt_a[0:64, :])
                g = persist.tile([d, 1], FP, tag=f"g{l}", name=f"g{l}")
                nc.vector.reduce_max(g[:], nxt_a[0:64, :], axis=mybir.AxisListType.X)
            else:
                x4 = persist.tile([128, NPTS], FP, name="x4")
                nc.vector.tensor_scalar_add(x4[:], zmax[:], b2_sb[:])
                g = persist.tile([128, 1], FP, tag="g3", name="g3")
                nc.vector.reduce_max(g[:], x4[:], axis=mybir.AxisListType.X)
            gtiles.append(g)

    # ---- head
    hp = ctx.enter_context(tc.tile_pool(name="head", bufs=1))
    hps = ctx.enter_context(tc.tile_pool(name="headps", bufs=1, space="PSUM"))

    gmat = hp.tile([128, 3], FP)
    emb_sb = hp.tile([64, 1], FP)
    nc.sync.dma_start(emb_sb[:], emb[:])
    nc.sync.dma_start(gmat[0:64, 0:1], gtiles[0][:])
    nc.sync.dma_start(gmat[64:128, 0:1], gtiles[1][:])
    nc.sync.dma_start(gmat[0:64, 1:2], gtiles[2][:])
    nc.sync.dma_start(gmat[64:128, 1:2], gtiles[3][0:64, :])
    nc.sync.dma_start(gmat[0:64, 2:3], gtiles[3][64:128, :])
    nc.sync.dma_start(gmat[64:128, 2:3], emb_sb[:])

    # enc: h[1,512] = g @ encw + encb, relu
    encw_sb = hp.tile([128, 3 * 512], FP)
    in_ap = bass.AP(encw[:].tensor, offset=0, ap=[[512, 128], [512 * 128, 3], [1, 512]])
    nc.sync.dma_start(encw_sb[:], in_ap)
    encb_sb = hp.tile([1, 512], FP)
    nc.sync.dma_start(encb_sb[:], encb[:])
    h_ps = hps.tile([1, 512], FP, tag="hps")
    for a in range(3):
        _mm(nc, h_ps[:], gmat[:, a: a + 1], encw_sb[:, 512 * a: 512 * (a + 1)],
            start=(a == 0), stop=(a == 2))
    h_sb = hp.tile([1, 512], FP)
    nc.vector.tensor_tensor(h_sb[:], h_ps[:], encb_sb[:], mybir.AluOpType.add)
    h_r = hp.tile([1, 512], FP)
    nc.scalar.activation(h_r[:], h_sb[:], mybir.ActivationFunctionType.Relu)
    # reshape h [1,512] -> [128, 4] chunks
    h_c = hp.tile([128, 4], FP)
    for a in range(4):
        nc.sync.dma_start(h_c[:, a: a + 1], h_r[:, 128 * a: 128 * (a + 1)])

    # dec1: d1[1,1024] = h @ decw1 + decb1, relu
    dw1_sb = hp.tile([128, 4 * 1024], FP)
    in_ap = bass.AP(decw1[:].tensor, offset=0,
                    ap=[[1024, 128], [1024 * 128, 4], [1, 1024]])
    nc.sync.dma_start(dw1_sb[:], in_ap)
    db1_sb = hp.tile([1, 1024], FP)
    nc.sync.dma_start(db1_sb[:], decb1[:])
    d1_ps = hps.tile([1, 1024], FP, tag="hps")
    for jj in range(2):
        for a in range(4):
            _mm(nc, d1_ps[:, 512 * jj: 512 * (jj + 1)], h_c[:, a: a + 1],
                dw1_sb[:, 1024 * a + 512 * jj: 1024 * a + 512 * (jj + 1)],
                start=(a == 0), stop=(a == 3))
    d1_sb = hp.tile([1, 1024], FP)
    nc.vector.tensor_tensor(d1_sb[:], d1_ps[:], db1_sb[:], mybir.AluOpType.add)
    d1_r = hp.tile([1, 1024], FP)
    nc.scalar.activation(d1_r[:], d1_sb[:], mybir.ActivationFunctionType.Relu)
    d1_c = hp.tile([128, 8], FP)
    for a in range(8):
        nc.sync.dma_start(d1_c[:, a: a + 1], d1_r[:, 128 * a: 128 * (a + 1)])

    # dec2: out[1,3072] = d1 @ decw2 + decb2 (two half-loads of decw2)
    db2_sb = hp.tile([1, 3072], FP)
    nc.sync.dma_start(db2_sb[:], decb2[:])
    o_sb = hp.tile([1, 3072], FP)
    o_ps = hps.tile([1, 3072], FP, tag="hps")
    with tc.tile_pool(name="dw2", bufs=1) as dwp:
        for half in range(2):
            dw2h = dwp.tile([128, 4 * 3072], FP, tag="dw2h")
            in_ap = bass.AP(decw2[:].tensor, offset=half * 4 * 128 * 3072,
                            ap=[[3072, 128], [3072 * 128, 4], [1, 3072]])
            nc.sync.dma_start(dw2h[:], in_ap)
            for jj in range(6):
                for a in range(4):
                    _mm(nc, o_ps[:, 512 * jj: 512 * (jj + 1)],
                        d1_c[:, 4 * half + a: 4 * half + a + 1],
                        dw2h[:, 3072 * a + 512 * jj: 3072 * a + 512 * (jj + 1)],
                        start=(half == 0 and a == 0), stop=(half == 1 and a == 3))
    nc.vector.tensor_tensor(o_sb[:], o_ps[:], db2_sb[:], mybir.AluOpType.add)
    out_flat = bass.AP(out.tensor, offset=0, ap=[[1, 3072]])
    nc.sync.dma_start(out_flat, o_sb[:])


_prog_cache = {}


def _get_program():
    key = str(MM_DT)
    if key not in _prog_cache:
        _prog_cache[key] = build_program()
    return _prog_cache[key]


def _host_prep(inputs):
    """Build per-core input maps from the full-problem inputs."""
    pos = np.asarray(inputs["pos"], np.float32)          # [8, 2048, 3]
    tooth_n = np.asarray(inputs["tooth_n"])              # [8]
    emb_all = (np.asarray(inputs["emb_table"], np.float32)[tooth_n]
               @ np.asarray(inputs["conv_emb_w"], np.float32)
               + np.asarray(inputs["conv_emb_b"], np.float32))  # [8, 64]

    shared = {}
    for l, pre in enumerate(["c1", "c2", "c3", "c4"]):
        w1 = np.asarray(inputs[f"{pre}_w1"], np.float32)
        din = w1.shape[0] // 2
        d = DOUT[l]
        w1a, w1b = w1[:din], w1[din:]
        pad = lambda m: np.pad(m, ((0, 64 - din), (0, 0)))
        shared[f"w1d{l}"] = np.ascontiguousarray(pad(w1a - w1b))
        shared[f"w1b{l}"] = np.ascontiguousarray(pad(w1b))
        shared[f"b1{l}"] = np.asarray(inputs[f"{pre}_b1"], np.float32).reshape(d, 1)
        shared[f"w2{l}"] = np.asarray(inputs[f"{pre}_w2"], np.float32)
        shared[f"b2{l}"] = np.asarray(inputs[f"{pre}_b2"], np.float32).reshape(d, 1)
    shared["ident"] = np.eye(128, dtype=np.float32)
    shared["encw"] = np.asarray(inputs["enc_w"], np.float32)
    shared["encb"] = np.asarray(inputs["enc_b"], np.float32).reshape(1, 512)
    shared["decw1"] = np.asarray(inputs["dec_w1"], np.float32)
    shared["decb1"] = np.asarray(inputs["dec_b1"], np.float32).reshape(1, 1024)
    shared["decw2"] = np.asarray(inputs["dec_w2"], np.float32)
    shared["decb2"] = np.asarray(inputs["dec_b2"], np.float32).reshape(1, 3072)

    in_maps = []
    for c in range(NCORES):
        m = dict(shared)
        pt = np.zeros((64, NPTS), np.float32)
        pt[0:3, :] = pos[c].T
        m["posT"] = pt
        m["emb"] = emb_all[c].reshape(64, 1)
        in_maps.append(m)
    return in_maps


def _run(inputs, trace=False, **kw):
    nc = _get_program()
    in_maps = _host_prep(inputs)
    res = run_bass_kernel_spmd(nc, in_maps, core_ids=list(range(NCORES)),
                               trace=trace, **kw)
    out = np.stack([res.results[c]["out"] for c in range(NCORES)], axis=0)
    return out, res


def kernel(**inputs) -> np.ndarray:
    return _run(inputs)[0]


if __name__ == "__main__":
    # smoke-test program build
    nc = build_program()
    print("program built ok")



# revision 10
# speedup vs baseline: 4.0188x; 4.0188x over previous
"""DGCNN generator kernel for Trainium2 (Bass/Tile), data-parallel over 8 NeuronCores.

Each core processes one point cloud (B=8). Per conv layer:
  - pairwise-distance surrogate S = X^T X - sq_j/2 via PE matmuls (K=65 augmented, fp32r)
  - exact top-16 candidates per point via DVE max8/match_replace/max_index on a bf16
    copy of the distance row (act-engine PSUM->SBUF cast)
  - neighbor feature gather via GPSIMD ap_gather in bf16, split in two k-halves so the
    edge MLP of half 1 overlaps the gather of half 2
  - edge MLP relu(A_i + B_j) @ w2 in bf16 (fp32 PSUM accumulate), max over the 10 neighbors
Point order is re-permuted every conv (sigma(c) = 16*(c%128) + c//128) to make the
index-relayout DMAs affine; all downstream stages are permutation-invariant.
Weights ship bf16 in one packed tensor (wpack16) + small fp32 sidecar (wpack32) to
minimize per-call host->device bytes; the big head weights prefetch at t=0.
"""
import sys
sys.path.insert(0, "/opt/trn_rl_repo")

import numpy as np
from contextlib import ExitStack

import concourse.bass as bass
import concourse.bacc as bacc
import concourse.tile as tile
from concourse.tile import add_dep_helper
from concourse import mybir
from concourse.bass_utils import run_bass_kernel_spmd

FP = mybir.dt.float32
BF = mybir.dt.bfloat16
F16 = mybir.dt.float16
U16 = mybir.dt.uint16
I16 = mybir.dt.int16
F32R = mybir.dt.float32r

NPTS = 2048
NCORES = 8
KNN = 10
NT = 16            # 128-row tiles per conv
DOUT = [64, 64, 64, 128]
NEG = -1e30

# ---- packed-weight element offsets (host layout must match) ----
def _pack_offsets():
    o32, o16 = {}, {}
    c = 0
    o32["emb"] = c; c += 64
    for l, d in enumerate(DOUT):
        o32[f"b1{l}"] = c; c += d
        o32[f"b2{l}"] = c; c += d
    o32["encb"] = c; c += 512
    o32["decb1"] = c; c += 1024
    o32["decb2"] = c; c += 3072
    n32 = c
    c = 0
    for l, d in enumerate(DOUT):
        o16[f"w2{l}"] = c; c += d * d
        o16[f"w1d{l}"] = c; c += 64 * d
        o16[f"w1b{l}"] = c; c += 64 * d
    o16["encw"] = c; c += 384 * 512
    o16["decw1"] = c; c += 512 * 1024
    o16["decw2"] = c; c += 1024 * 3072
    n16 = c
    return o32, n32, o16, n16

OFF32, N32, OFF16, N16 = _pack_offsets()


def _mm(nc, out, lhsT, rhs, **kw):
    return nc.tensor.matmul(out, lhsT, rhs, **kw)


def build_program():
    nc = bacc.Bacc("TRN2", target_bir_lowering=False, debug=False,
                   enable_asserts=False, num_devices=NCORES)

    posT = nc.dram_tensor("posT", [3, NPTS], FP, kind="ExternalInput").ap()
    identh = nc.dram_tensor("identh", [128, 128], F16, kind="ExternalInput").ap()
    wpack32 = nc.dram_tensor("wpack32", [1, N32], FP, kind="ExternalInput").ap()
    wpack16 = nc.dram_tensor("wpack16", [1, N16], BF, kind="ExternalInput").ap()
    out = nc.dram_tensor("out", [1024, 3], FP, kind="ExternalOutput").ap()

    io = dict(posT=posT, identh=identh, wpack32=wpack32, wpack16=wpack16, out=out)
    with tile.TileContext(nc) as tc, ExitStack() as ctx:
        _body(ctx, tc, io)
    nc.compile()
    return nc


def _w32(t, name, rows, cols):
    return bass.AP(t.tensor, offset=OFF32[name], ap=[[cols, rows], [1, cols]])


def _w16(t, name, rows, cols):
    return bass.AP(t.tensor, offset=OFF16[name], ap=[[cols, rows], [1, cols]])


def _body(ctx, tc, io):
    nc = tc.nc
    posT, identh, out = io["posT"], io["identh"], io["out"]
    wp32, wp16 = io["wpack32"], io["wpack16"]

    persist = ctx.enter_context(tc.tile_pool(name="persist", bufs=1))

    # ping-pong feature tensors (fp32) + per-layer bf16 copies with aug rows
    xa = [persist.tile([64, NPTS], FP, tag=f"xa{i}", name=f"xa{i}") for i in range(2)]
    abf = persist.tile([65, NPTS], BF, name="abf")   # row 64: ones
    cbf = persist.tile([65, NPTS], BF, name="cbf")   # row 64: -sq/2
    nc.vector.memset(abf[64:65, :], 1.0)
    # layer-0 input: zero rows 0:64, then pos DMA overwrites rows 0:3
    nc.vector.memset(xa[0][0:64, :], 0.0)
    xa_writer = [None, None]
    xa_writer[0] = nc.sync.dma_start(xa[0][0:3, :], posT[:])

    ones64 = persist.tile([64, 1], BF)
    nc.vector.memset(ones64[:], 1.0)
    ident = persist.tile([128, 128], F16)
    nc.sync.dma_start(ident[:], identh[:])

    # ---- head-weight prefetch (bf16, hidden behind the conv layers)
    headw = ctx.enter_context(tc.tile_pool(name="headw", bufs=1))
    encw_sb = headw.tile([128, 3 * 512], BF)
    nc.scalar.dma_start(encw_sb[:], bass.AP(
        wp16.tensor, offset=OFF16["encw"],
        ap=[[512, 128], [512 * 128, 3], [1, 512]]))
    dw1_sb = headw.tile([128, 4 * 1024], BF)
    nc.scalar.dma_start(dw1_sb[:], bass.AP(
        wp16.tensor, offset=OFF16["decw1"],
        ap=[[1024, 128], [1024 * 128, 4], [1, 1024]]))
    dw2_sb = headw.tile([128, 8 * 3072], BF)
    nc.gpsimd.dma_start(dw2_sb[:], bass.AP(
        wp16.tensor, offset=OFF16["decw2"],
        ap=[[3072, 128], [3072 * 128, 8], [1, 3072]]))


    gtiles = []   # per-conv global-max [d, 1]

    dmaq = [nc.sync, nc.scalar, nc.gpsimd]

    for l in range(4):
        d = DOUT[l]
        cur_a = xa[l % 2]
        with ExitStack() as cctx:
            wpool = cctx.enter_context(tc.tile_pool(name=f"w{l}", bufs=1))
            # load weights for this conv
            w1d_sb = wpool.tile([64, d], BF)
            nc.sync.dma_start(w1d_sb[:], _w16(wp16, f"w1d{l}", 64, d))
            w1b_sb = wpool.tile([64, d], BF)
            nc.sync.dma_start(w1b_sb[:], _w16(wp16, f"w1b{l}", 64, d))
            b1_sb = wpool.tile([d, 1], FP)
            nc.sync.dma_start(b1_sb[:], _w32(wp32, f"b1{l}", d, 1))
            w2_sb = wpool.tile([d, d], BF)
            nc.sync.dma_start(w2_sb[:], _w16(wp16, f"w2{l}", d, d))
            b2_sb = wpool.tile([d, 1], FP)
            nc.sync.dma_start(b2_sb[:], _w32(wp32, f"b2{l}", d, 1))

            # ---- -sq/2 into cur_b row 64
            with tc.tile_pool(name=f"sq{l}", bufs=2, space="PSUM") as ps_sq, \
                 tc.tile_pool(name=f"sqs{l}", bufs=1) as sq_sb:
                x2 = sq_sb.tile([64, NPTS], BF, tag="x2")
                nc.vector.scalar_tensor_tensor(
                    out=x2[:], in0=cur_a[0:64, :], scalar=-0.5, in1=cur_a[0:64, :],
                    op0=mybir.AluOpType.mult, op1=mybir.AluOpType.mult)
                for j in range(4):
                    sq_ps = ps_sq.tile([1, 512], FP, tag="sq")
                    _mm(nc, sq_ps[:], ones64[:], x2[:, 512 * j: 512 * (j + 1)],
                        start=True, stop=True)
                    nc.scalar.copy(cbf[64:65, 512 * j: 512 * (j + 1)], sq_ps[:])
                abf_w = nc.scalar.copy(abf[0:64, :], cur_a[0:64, :])
                nc.scalar.copy(cbf[0:64, :], cur_a[0:64, :])

            # ---- phase B: distances + top-16 per 128-row tile (bf16 scans)
            iia = persist.tile([128, 128], U16, name=f"iia{l}")
            iib = persist.tile([128, 128], U16, name=f"iib{l}")
            with tc.tile_pool(name=f"s{l}", bufs=2, space="PSUM") as ps_s, \
                 tc.tile_pool(name=f"sb{l}", bufs=2) as sbf_pool, \
                 tc.tile_pool(name=f"c{l}", bufs=3) as cpool:
                for m in range(NT):
                    s_ps = ps_s.tile([128, NPTS], FP, tag="s")
                    for j in range(4):
                        _mm(nc, s_ps[:, 512 * j: 512 * (j + 1)],
                            abf[:, 128 * m: 128 * (m + 1)],
                            cbf[:, 512 * j: 512 * (j + 1)], start=True, stop=True)
                    s_bf = sbf_pool.tile([128, NPTS], BF, tag="sbf")
                    nc.scalar.copy(s_bf[:], s_ps[:])
                    c = cpool.tile([128, 64], BF, tag="c")
                    for ch in range(8):
                        nc.vector.max(c[:, 8 * ch: 8 * ch + 8],
                                      s_bf[:, 256 * ch: 256 * (ch + 1)])
                    v1 = cpool.tile([128, 8], BF, tag="v1")
                    nc.vector.max(v1[:], c[:])
                    c2 = cpool.tile([128, 64], BF, tag="c2")
                    nc.vector.match_replace(c2[:], v1[:], c[:], NEG)
                    v2 = cpool.tile([128, 8], BF, tag="v2")
                    nc.vector.max(v2[:], c2[:])
                    ii = iia if m < 8 else iib
                    t = m % 8
                    nc.vector.max_index(ii[:, 16 * t: 16 * t + 8], v1[:], s_bf[:])
                    nc.vector.max_index(ii[:, 16 * t + 8: 16 * t + 16], v2[:], s_bf[:])

            # ---- transpose + relayout into IDXW
            idxw = persist.tile([128, 1280], U16, name=f"idxw{l}")
            Ts = [persist.tile([128, 128], U16, tag=f"T{a}", name=f"T{l}{a}") for a in range(2)]
            with tc.tile_pool(name=f"tp{l}", bufs=2, space="PSUM") as ps_t, \
                 tc.tile_pool(name=f"tf{l}", bufs=2) as tfp:
                for a, ii in enumerate([iia, iib]):
                    iif = tfp.tile([128, 128], F16, tag="iif")
                    nc.vector.tensor_copy(
                        iif[:].rearrange("p (s t) -> p t s", s=16, t=8),
                        ii[:].rearrange("p (t s) -> p t s", t=8, s=16))
                    tp = ps_t.tile([128, 128], F16, tag="tp")
                    nc.tensor.transpose(tp[:], iif[:], ident[:])
                    nc.vector.tensor_copy(Ts[a][:], tp[:])
            # replicate indices into each 16-partition gpsimd group
            ngrp = d // 16
            isem = nc.alloc_semaphore(f"idxsem{l}")
            with tc.tile_critical():
                for a in range(2):
                    for k in range(KNN):
                        dmaq[(a * KNN + k) % 3].dma_start(
                            idxw[8 * a: 8 * a + 8, 128 * k: 128 * (k + 1)],
                            Ts[a][8 * k: 8 * k + 8, :]).then_inc(isem, 16)
                for rc in range(1, ngrp):
                    dmaq[rc % 3].dma_start(
                        idxw[16 * rc: 16 * rc + 16, :],
                        idxw[0:16, :])._wait_ge(isem, 320).then_inc(isem, 16)
                nc.sync.nop()._wait_ge(isem, 320 + 16 * (ngrp - 1))

            # ---- phase C: A (sigma-permuted), Bt in bf16
            bt = wpool.tile([d, NPTS], FP, name=f"bt{l}")
            a_sb = wpool.tile([d, NPTS], FP, name=f"a{l}")
            with tc.tile_pool(name=f"ab{l}", bufs=2, space="PSUM") as ps_ab:
                for j in range(4):
                    b_ps = ps_ab.tile([d, 512], FP, tag="ab")
                    _mm(nc, b_ps[:], w1b_sb[:],
                        abf[0:64, 512 * j: 512 * (j + 1)], start=True, stop=True)
                    nc.scalar.copy(bt[:, 512 * j: 512 * (j + 1)], b_ps[:])
                for j in range(4):
                    a_ps = ps_ab.tile([d, 512], FP, tag="ab")
                    rhs = abf[0:64, :].rearrange(
                        "c (q p) -> c p q", q=16, p=128)[:, 32 * j: 32 * (j + 1), :]
                    amm = _mm(nc, a_ps[:], w1d_sb[:], rhs, start=True, stop=True)
                    if amm is not None:
                        add_dep_helper(amm.ins, abf_w.ins,
                                       reason="perm-rhs read after abf write")
                    nc.vector.tensor_scalar_add(
                        a_sb[:, 512 * j: 512 * (j + 1)], a_ps[:], b1_sb[:])

            # ---- phase D/E: gather (two k-halves) + edge MLP + max over k
            zmax = wpool.tile([d, NPTS], FP, name=f"zmax{l}")
            KH = 2
            with tc.tile_pool(name=f"g{l}", bufs=2) as gpool, \
                 tc.tile_pool(name=f"z{l}", bufs=2, space="PSUM") as ps_z, \
                 tc.tile_pool(name=f"h{l}", bufs=2) as hpool:
                for half in range(KNN // KH):
                    bg = gpool.tile([d, KH * NPTS], FP, tag="bg")
                    nc.gpsimd.ap_gather(
                        bg[:], bt[:],
                        idxw[0:d, 128 * KH * half: 128 * KH * (half + 1)].bitcast(I16),
                        channels=d, num_elems=NPTS, d=1, num_idxs=KH * NPTS)
                    for kh in range(KH):
                        k = half * KH + kh
                        h1 = hpool.tile([d, NPTS], BF, tag="h1")
                        nc.vector.tensor_tensor(
                            h1[:], a_sb[:], bg[:, NPTS * kh: NPTS * (kh + 1)],
                            mybir.AluOpType.add)
                        h1r = hpool.tile([d, NPTS], BF, tag="h1r")
                        nc.scalar.activation(h1r[:], h1[:],
                                             mybir.ActivationFunctionType.Relu)
                        for jh in range(2):
                            z_ps = ps_z.tile([d, 1024], FP, tag="z")
                            for jj in range(2):
                                nc.tensor.matmul(
                                    z_ps[:, 512 * jj: 512 * (jj + 1)], w2_sb[:],
                                    h1r[:, 1024 * jh + 512 * jj: 1024 * jh + 512 * (jj + 1)],
                                    start=True, stop=True)
                            if k == 0:
                                nc.scalar.copy(zmax[:, 1024 * jh: 1024 * (jh + 1)], z_ps[:])
                            else:
                                nc.vector.tensor_tensor(
                                    zmax[:, 1024 * jh: 1024 * (jh + 1)],
                                    zmax[:, 1024 * jh: 1024 * (jh + 1)], z_ps[:],
                                    mybir.AluOpType.max)

            # ---- x_next = zmax + b2 ; g_l
            if l < 3:
                nxt_a = xa[(l + 1) % 2]
                xa_writer[(l + 1) % 2] = nc.vector.tensor_scalar_add(
                    nxt_a[0:64, :], zmax[:], b2_sb[:])
                g = persist.tile([d, 1], FP, tag=f"g{l}", name=f"g{l}")
                nc.vector.reduce_max(g[:], nxt_a[0:64, :], axis=mybir.AxisListType.X)
            else:
                gm = persist.tile([128, 1], FP, tag="gm3", name="gm3")
                nc.vector.reduce_max(gm[:], zmax[:], axis=mybir.AxisListType.X)
                g = persist.tile([128, 1], FP, tag="g3", name="g3")
                nc.vector.tensor_scalar_add(g[:], gm[:], b2_sb[:])
            gtiles.append(g)

    # ---- head (bf16 weights, prefetched at t=0)
    hp = ctx.enter_context(tc.tile_pool(name="head", bufs=1))
    hps = ctx.enter_context(tc.tile_pool(name="headps", bufs=1, space="PSUM"))

    gmat32 = hp.tile([128, 3], FP)
    emb_sb = hp.tile([64, 1], FP)
    nc.sync.dma_start(emb_sb[:], _w32(wp32, "emb", 64, 1))
    nc.sync.dma_start(gmat32[0:64, 0:1], gtiles[0][:])
    nc.sync.dma_start(gmat32[64:128, 0:1], gtiles[1][:])
    nc.sync.dma_start(gmat32[0:64, 1:2], gtiles[2][:])
    nc.sync.dma_start(gmat32[64:128, 1:2], gtiles[3][0:64, :])
    nc.sync.dma_start(gmat32[0:64, 2:3], gtiles[3][64:128, :])
    nc.sync.dma_start(gmat32[64:128, 2:3], emb_sb[:])
    gmat = hp.tile([128, 3], BF)
    nc.vector.tensor_copy(gmat[:], gmat32[:])

    # enc: h[1,512] = g @ encw + encb, relu
    encb_sb = hp.tile([1, 512], FP)
    nc.sync.dma_start(encb_sb[:], _w32(wp32, "encb", 1, 512))
    h_ps = hps.tile([1, 512], FP, tag="hps")
    for a in range(3):
        nc.tensor.matmul(h_ps[:], gmat[:, a: a + 1],
                         encw_sb[:, 512 * a: 512 * (a + 1)],
                         start=(a == 0), stop=(a == 2))
    h_sb = hp.tile([1, 512], FP)
    nc.vector.tensor_tensor(h_sb[:], h_ps[:], encb_sb[:], mybir.AluOpType.add)
    h_r = hp.tile([1, 512], BF)
    nc.scalar.activation(h_r[:], h_sb[:], mybir.ActivationFunctionType.Relu)
    h_c = hp.tile([128, 4], BF)
    for a in range(4):
        nc.sync.dma_start(h_c[:, a: a + 1], h_r[:, 128 * a: 128 * (a + 1)])

    # dec1: d1[1,1024] = h @ decw1 + decb1, relu
    db1_sb = hp.tile([1, 1024], FP)
    nc.sync.dma_start(db1_sb[:], _w32(wp32, "decb1", 1, 1024))
    d1_ps = hps.tile([1, 1024], FP, tag="hps")
    for jj in range(2):
        for a in range(4):
            nc.tensor.matmul(
                d1_ps[:, 512 * jj: 512 * (jj + 1)], h_c[:, a: a + 1],
                dw1_sb[:, 1024 * a + 512 * jj: 1024 * a + 512 * (jj + 1)],
                start=(a == 0), stop=(a == 3))
    d1_sb = hp.tile([1, 1024], FP)
    nc.vector.tensor_tensor(d1_sb[:], d1_ps[:], db1_sb[:], mybir.AluOpType.add)
    d1_r = hp.tile([1, 1024], BF)
    nc.scalar.activation(d1_r[:], d1_sb[:], mybir.ActivationFunctionType.Relu)
    d1_c = hp.tile([128, 8], BF)
    for a in range(8):
        nc.sync.dma_start(d1_c[:, a: a + 1], d1_r[:, 128 * a: 128 * (a + 1)])

    # dec2: out[1,3072] = d1 @ decw2 + decb2 (fully resident, prefetched)
    db2_sb = hp.tile([1, 3072], FP)
    nc.sync.dma_start(db2_sb[:], _w32(wp32, "decb2", 1, 3072))
    o_sb = hp.tile([1, 3072], FP)
    o_ps = hps.tile([1, 3072], FP, tag="hps")
    for jj in range(6):
        for a in range(8):
            nc.tensor.matmul(
                o_ps[:, 512 * jj: 512 * (jj + 1)], d1_c[:, a: a + 1],
                dw2_sb[:, 3072 * a + 512 * jj: 3072 * a + 512 * (jj + 1)],
                start=(a == 0), stop=(a == 7))
    nc.vector.tensor_tensor(o_sb[:], o_ps[:], db2_sb[:], mybir.AluOpType.add)
    out_flat = bass.AP(out.tensor, offset=0, ap=[[1, 3072]])
    nc.sync.dma_start(out_flat, o_sb[:])


_prog_cache = {}


def _get_program():
    if "p" not in _prog_cache:
        _prog_cache["p"] = build_program()
    return _prog_cache["p"]


def _host_prep(inputs):
    """Build per-core input maps from the full-problem inputs."""
    from ml_dtypes import bfloat16
    pos = np.asarray(inputs["pos"], np.float32)          # [8, 2048, 3]
    tooth_n = np.asarray(inputs["tooth_n"])              # [8]
    emb_all = (np.asarray(inputs["emb_table"], np.float32)[tooth_n]
               @ np.asarray(inputs["conv_emb_w"], np.float32)
               + np.asarray(inputs["conv_emb_b"], np.float32))  # [8, 64]

    w32 = np.zeros(N32, np.float32)
    w16 = np.zeros(N16, bfloat16)

    def put32(name, arr):
        a = np.asarray(arr, np.float32).reshape(-1)
        w32[OFF32[name]: OFF32[name] + a.size] = a

    def put16(name, arr):
        a = np.asarray(arr, np.float32).reshape(-1)
        w16[OFF16[name]: OFF16[name] + a.size] = a.astype(bfloat16)

    for l, pre in enumerate(["c1", "c2", "c3", "c4"]):
        w1 = np.asarray(inputs[f"{pre}_w1"], np.float32)
        din = w1.shape[0] // 2
        w1a, w1b = w1[:din], w1[din:]
        pad = lambda m: np.pad(m, ((0, 64 - din), (0, 0)))
        put16(f"w1d{l}", pad(w1a - w1b))
        put16(f"w1b{l}", pad(w1b))
        put32(f"b1{l}", inputs[f"{pre}_b1"])
        put32(f"b2{l}", inputs[f"{pre}_b2"])
        put16(f"w2{l}", inputs[f"{pre}_w2"])
    put32("encb", inputs["enc_b"])
    put32("decb1", inputs["dec_b1"])
    put32("decb2", inputs["dec_b2"])
    put16("encw", inputs["enc_w"])
    put16("decw1", inputs["dec_w1"])
    put16("decw2", inputs["dec_w2"])

    identh = np.eye(128, dtype=np.float16)

    in_maps = []
    for c in range(NCORES):
        m = {"wpack16": w16.reshape(1, N16), "identh": identh,
             "posT": np.ascontiguousarray(pos[c].T)}
        wc = w32.copy()
        wc[OFF32["emb"]: OFF32["emb"] + 64] = emb_all[c]
        m["wpack32"] = wc.reshape(1, N32)
        in_maps.append(m)
    return in_maps


def _run(inputs, trace=False, **kw):
    nc = _get_program()
    in_maps = _host_prep(inputs)
    res = run_bass_kernel_spmd(nc, in_maps, core_ids=list(range(NCORES)),
                               trace=trace, **kw)
    out = np.stack([res.results[c]["out"] for c in range(NCORES)], axis=0)
    return out, res


def kernel(**inputs) -> np.ndarray:
    return _run(inputs)[0]


if __name__ == "__main__":
    nc = build_program()
    print("program built ok")
